# revision 1
# baseline (speedup 1.0000x reference)
"""Trainium2 Bass kernel for nn_CazzyLoss (multi-component loss).

Strategy (8 NeuronCores, data parallel):
  - disease CE / risk CE / time loss / uncertainty: rows (B*S=16384) sharded
    2048 per core (= one batch element per core). Logits are shipped fp16
    (halves the dominant HBM stream); exp+accum runs on the ACT engine,
    which is the critical engine (~2.9M exp elements / 128 lanes).
  - survival concordance (n=4096 pairwise): core c owns rows i = c + 8k
    (k=0..511, k on the free dim; j on partitions, 32 j-tiles of 128).
    Time comparisons use host-built fp16 *rank keys* (strictly increasing
    fp16 enumeration of the sorted order, event fold to +big) so fp16
    compares are exact; curve-mean comparisons use centered fp16 (ties
    only within ~1 fp16 ulp of 0 -> negligible count error).
    Prefix region (full 16-column blocks i<j) is compare+count via
    tensor_scalar (4x DVE mode) + ones-matmul PSUM accumulation on PE;
    AB (concordant) products split between DVE and Pool engines, reduced
    by a second PE matmul group. The per-core diagonal band uses a
    constant mask as before.
  - i-side replicates (t-keys, own-means) reach all 128 partitions via
    [1,512] rows broadcast by a K=1 ones-matmul on PE (no big DMA
    roundtrip); own-means take a tiny [P,4] DRAM roundtrip first.
  - target-logit gather: one batched indirect DMA on gpsimd.
  - single batched Ln over [sumexp | risk sumexp | rates] closes CE/time.
  - Each core emits 8 partial sums; the host combines them into the [6]
    output (pure O(1) scalar arithmetic).

Host-side work is layout-only: slicing, reshapes, dtype casts, rank/index
tables and constant masks. All tensor arithmetic runs on device.
"""

import numpy as np

B, S, VOCAB = 8, 2048, 1400
N_SURV, T_SURV = 4096, 120
NCORES = 8
P = 128
RT = S // P          # 16 row-tiles per core
G2 = 2               # row-tiles per logits DMA
NLT = RT // G2       # 8 logits tiles
NJT = N_SURV // P    # 32 j-tiles
W = N_SURV // NCORES # 512 i's per core
WMAX = 16 * (NJT - 1)  # widest prefix = 496
BW = 16              # band width (128/8)
EPS = 1e-6
BIGKEY = np.float16(60000.0)

# smalls (f32) column offsets
O_RTGT, O_TTE, O_TTG, O_UNC = 0, 16, 32, 48
O_IOTA5, O_RISKL, O_KEYTAB = 64, 144, 224
SMALLW = 256
# smalls16 (fp16) column offsets
O_BMASK = 0
SMALL16W = 16

# output columns
C_NBAND, C_SBAND, C_CE, C_RISK, C_TIME, C_UNC = range(6)
NOUT = 6

_CACHE = {}


def _build_nc():
    import concourse.bass as bass
    import concourse.bacc as bacc
    import concourse.tile as tile
    from concourse import mybir
    from contextlib import ExitStack

    f32 = mybir.dt.float32
    fp16 = mybir.dt.float16
    i32 = mybir.dt.int32
    Alu = mybir.AluOpType
    Act = mybir.ActivationFunctionType
    AxX = mybir.AxisListType.X

    nc = bacc.Bacc(None)

    logits_h = nc.declare_dram_parameter("logits", [S, VOCAB], fp16, isOutput=False)
    gidx_h = nc.declare_dram_parameter("gidx", [P, RT], i32, isOutput=False)
    smalls_h = nc.declare_dram_parameter("smalls", [P, SMALLW], f32, isOutput=False)
    smalls16_h = nc.declare_dram_parameter("smalls16", [P, SMALL16W], fp16,
                                           isOutput=False)
    keyf_h = nc.declare_dram_parameter("keyf", [1, W], fp16, isOutput=False)
    curvo_h = nc.declare_dram_parameter("curvo", [P, 4 * T_SURV], fp16,
                                        isOutput=False)
    curves_h = nc.declare_dram_parameter("curves", [P, NJT * T_SURV], fp16,
                                         isOutput=False)
    out_h = nc.declare_dram_parameter("partials", [1, NOUT], f32, isOutput=True)
    pref_h = nc.declare_dram_parameter("prefix", [1, 2 * WMAX], f32,
                                       isOutput=True)

    with tile.TileContext(nc) as tc, ExitStack() as ctx:
        io = ctx.enter_context(tc.tile_pool(name="io", bufs=1))
        lp = ctx.enter_context(tc.tile_pool(name="lp", bufs=1))
        esc = ctx.enter_context(tc.tile_pool(name="esc", bufs=2))
        abd = ctx.enter_context(tc.tile_pool(name="abd", bufs=6))
        abp = ctx.enter_context(tc.tile_pool(name="abp", bufs=3))
        dpool = ctx.enter_context(tc.tile_pool(name="dram", bufs=1, space="DRAM"))
        psum = ctx.enter_context(tc.tile_pool(name="psum", bufs=1, space="PSUM"))

        partials = io.tile([P, NOUT], f32)

        # ---------- DMA stream ------
        # Issues split across the SP and Pool DGE queues: one queue's
        # ~660ns-per-issue serialization otherwise throttles the stream
        # start (keyf/curves land late and push the whole sweep into the
        # tail). Each queue is FIFO-chained for deterministic landing order.
        from concourse.tile_rust import add_dep_helper
        sp_chain = []
        gp_chain = []

        def sp_dma(out, in_):
            bi = nc.sync.dma_start(out=out, in_=in_)
            if sp_chain:
                add_dep_helper(bi.ins, sp_chain[-1].ins, sync=False,
                               reason="DMA issue/data ordering")
            sp_chain.append(bi)
            return bi

        def gp_dma(out, in_):
            bi = nc.gpsimd.dma_start(out=out, in_=in_)
            if gp_chain:
                add_dep_helper(bi.ins, gp_chain[-1].ins, sync=False,
                               reason="DMA issue/data ordering")
            gp_chain.append(bi)
            return bi

        logits_r1 = logits_h[:].rearrange("(t p) c -> t p c", p=P)
        logits_rp = logits_h[:][2 * P:, :].rearrange(
            "(a q p) c -> a p q c", q=G2, p=P)
        L0a = lp.tile([P, 1, VOCAB], fp16, tag="L0a")
        L0b = lp.tile([P, 1, VOCAB], fp16, tag="L0b")
        Ltiles = []
        for a in range(NLT - 1):
            L = lp.tile([P, G2, VOCAB], fp16, tag=f"L{a}")
            Ltiles.append(L)

        gidx_t = io.tile([P, RT], i32, tag="gidx")
        s16 = io.tile([P, SMALL16W], fp16, tag="s16")
        keyf_t = io.tile([1, W], fp16, tag="keyf")
        curvo = io.tile([P, 4 * T_SURV], fp16, tag="curvo")
        smalls_t = io.tile([P, SMALLW], f32, tag="smalls")
        call = io.tile([P, NJT * T_SURV], fp16, tag="call")
        HALF = 16 * T_SURV

        # Pool queue: the small early inputs + odd L tiles
        gp_dma(keyf_t[:], keyf_h[:])
        gp_dma(curvo[:], curvo_h[:])
        gp_dma(L0b[:, 0, :], logits_r1[1])
        gp_dma(gidx_t[:], gidx_h[:])
        gp_dma(s16[:], smalls16_h[:])
        gp_dma(Ltiles[1][:], logits_rp[1])
        gp_dma(call[:, HALF:2 * HALF], curves_h[:, HALF:2 * HALF])
        gp_dma(Ltiles[3][:], logits_rp[3])
        gp_dma(Ltiles[5][:], logits_rp[5])

        # SP queue: even L tiles + the rest
        sp_dma(L0a[:, 0, :], logits_r1[0])
        sp_dma(Ltiles[0][:], logits_rp[0])
        sp_dma(smalls_t[:], smalls_h[:])
        sp_dma(Ltiles[2][:], logits_rp[2])
        sp_dma(call[:, 0:HALF], curves_h[:, 0:HALF])
        sp_dma(Ltiles[4][:], logits_rp[4])
        sp_dma(Ltiles[6][:], logits_rp[6])
        call3 = call[:].rearrange("p (j t) -> p j t", t=T_SURV)

        bm16 = s16[:, O_BMASK:O_BMASK + BW]
        rtgtf = smalls_t[:, O_RTGT:O_RTGT + RT]
        tte = smalls_t[:, O_TTE:O_TTE + RT]
        ttg = smalls_t[:, O_TTG:O_TTG + RT]
        unc = smalls_t[:, O_UNC:O_UNC + RT]
        iota5 = smalls_t[:, O_IOTA5:O_IOTA5 + 80].rearrange("p (a b) -> p a b", b=5)
        keytab = smalls_t[:, O_KEYTAB:O_KEYTAB + NJT]
        riskl = smalls_t[:, O_RISKL:O_RISKL + 80].rearrange("p (a b) -> p a b", b=5)

        # ---------- constants + ACT exp-table warm ----------
        dummy1 = io.tile([P, 1], f32)
        nc.vector.memset(dummy1[:], 1.0)
        ones16 = io.tile([1, P], fp16)
        nc.vector.memset(ones16[:], 1.0)
        onesc = io.tile([P, 1], fp16)
        nc.vector.memset(onesc[:], 1.0)
        warmact = io.tile([P, 1], f32)
        nc.scalar.activation(out=warmact[:], in_=dummy1[:], func=Act.Exp)
        epsb = io.tile([P, 1], f32)
        nc.vector.memset(epsb[:], EPS)

        # ---------- disease CE exp stream (ACT) ----------
        fin = io.tile([P, 48], f32)    # sumexp | risk sumexp | rates
        finl = io.tile([P, 48], f32)
        other = io.tile([P, 48], f32)  # l_target | risk l_target | rate*tgt

        # own-means -> [P,4] (i-side m replicate source)
        m_own = io.tile([P, 4], f32)
        nc.vector.tensor_reduce(
            out=m_own[:], in_=curvo[:].rearrange("p (q t) -> p q t", t=T_SURV),
            axis=AxX, op=Alu.add)
        m_ownc = io.tile([P, 4], fp16)
        nc.vector.tensor_scalar_add(out=m_ownc[:], in0=m_own[:], scalar1=-60.0)
        md = dpool.tile([P, 4], fp16)
        m_row = io.tile([1, W], fp16)

        exp_bis = []
        row_srcs = [L0a[:, 0, :], L0b[:, 0, :]]
        for a in range(NLT - 1):
            for q in range(G2):
                row_srcs.append(Ltiles[a][:, q, :])
        for t, src_ap in enumerate(row_srcs):
            E = esc.tile([P, VOCAB], fp16, tag="E")
            bi = nc.scalar.activation(
                out=E[:], in_=src_ap, func=Act.Exp,
                accum_out=fin[:, t:t + 1],
            )
            exp_bis.append(bi)
            if t == 2:
                bi_md = nc.scalar.dma_start(out=md[:], in_=m_ownc[:])
            if t == 3:
                bi_mr = nc.scalar.dma_start(
                    out=m_row[:],
                    in_=md[:].rearrange("p q -> (p q)")[None, :])
                add_dep_helper(bi_mr.ins, bi_md.ins, sync=False,
                               reason="roundtrip order")
            if t == 9:
                # risk exp mid-stream (riskl lands ~2/3 in)
                rE = io.tile([P, RT, 5], f32)
                nc.scalar.activation(out=rE[:], in_=riskl, func=Act.Exp)

        # ---------- concordance: i-side replicates ----------
        # t-key broadcast (PE, K=1 ones matmul)
        psT = psum.tile([P, W], f32)
        nc.tensor.matmul(out=psT[:], lhsT=ones16[:], rhs=keyf_t[:],
                         start=True, stop=True)
        t_rep = io.tile([P, W], fp16)
        nc.vector.tensor_copy(out=t_rep[:], in_=psT[:])

        # ---------- A-matrix prefix sweep (DVE, 4x fp16 tensor_scalar) ------
        Atiles = {}
        for jt in range(NJT - 1, 0, -1):
            w = BW * jt
            A = io.tile([P, w], fp16, tag=f"A{jt}")
            nc.vector.tensor_scalar(
                out=A[:], in0=t_rep[:, :w], scalar1=keytab[:, jt:jt + 1],
                scalar2=None, op0=Alu.is_lt)
            Atiles[jt] = A

        # ---------- j-side means: high half on DVE, low half on Pool --------
        mtab_f = io.tile([P, NJT], f32)
        m_tabc = io.tile([P, NJT], f32)
        nc.vector.tensor_reduce(out=mtab_f[:, 16:32], in_=call3[:, 16:32, :],
                                axis=AxX, op=Alu.add)
        nc.vector.tensor_scalar_add(out=m_tabc[:, 16:32],
                                    in0=mtab_f[:, 16:32], scalar1=-60.0)

        # gather of target logits: batched indirect DMA on gpsimd
        lt16 = io.tile([P, RT], fp16)
        logits_flat = logits_h[:].rearrange("a b -> (a b)")[:, None]
        bi_g = nc.gpsimd.indirect_dma_start(
            out=lt16[:],
            out_offset=None,
            in_=logits_flat,
            in_offset=bass.IndirectOffsetOnAxis(ap=gidx_t[:], axis=0),
        )
        add_dep_helper(bi_g.ins, bi_mr.ins, sync=True,
                       reason="gather after m roundtrip (drain poisons queue)")

        nc.vector.tensor_reduce(out=mtab_f[:, 0:16], in_=call3[:, 0:16, :],
                                axis=AxX, op=Alu.add)
        nc.vector.tensor_scalar_add(out=m_tabc[:, 0:16],
                                    in0=mtab_f[:, 0:16], scalar1=-60.0)

        # m broadcast (PE) after the N-matmul block below (PE program order);
        # traced here for data deps, PE order set by trace order of matmuls.
        psM = psum.tile([P, W], f32)
        psN = psum.tile([1, WMAX], f32)
        psS = psum.tile([1, WMAX], f32)

        # N-matmuls (PE): count prefix-valid pairs from A tiles
        for jt in range(NJT - 1, 0, -1):
            w = BW * jt
            nc.tensor.matmul(out=psN[:, :w], lhsT=onesc[:], rhs=Atiles[jt][:],
                             start=(jt == NJT - 1), stop=(jt == 1))

        nc.tensor.matmul(out=psM[:], lhsT=ones16[:], rhs=m_row[:],
                         start=True, stop=True)
        m_rep = io.tile([P, W], fp16)
        nc.vector.tensor_copy(out=m_rep[:], in_=psM[:])

        prefsb = io.tile([1, 2 * WMAX], f32)
        nc.vector.tensor_copy(out=prefsb[:, 0:WMAX], in_=psN[:])

        # ---------- AB sweep (DVE STT; Pool has no vector ISA) --------------
        ab_of = {}
        for jt in range(NJT - 1, 0, -1):
            w = BW * jt
            AB = abd.tile([P, WMAX], fp16, tag="ABd")
            nc.vector.scalar_tensor_tensor(
                out=AB[:, :w], in0=m_rep[:, :w], scalar=m_tabc[:, jt:jt + 1],
                in1=Atiles[jt][:], op0=Alu.is_lt, op1=Alu.mult)
            ab_of[jt] = AB
            if jt == NJT - 3:
                # risk sumexp spliced early so the batched Ln never stalls
                rse_bi = nc.vector.tensor_reduce(out=fin[:, 16:32], in_=rE[:],
                                                 axis=AxX, op=Alu.add)

        # S-matmuls (PE)
        for jt in range(NJT - 1, 0, -1):
            w = BW * jt
            nc.tensor.matmul(out=psS[:, :w], lhsT=onesc[:],
                             rhs=ab_of[jt][:, :w],
                             start=(jt == NJT - 1), stop=(jt == 1))

        # ---------- diagonal band (DVE) ----------
        t_rep3 = t_rep[:].rearrange("p (a g) -> p a g", g=BW)
        m_rep3 = m_rep[:].rearrange("p (a g) -> p a g", g=BW)
        tj_b = keytab[:, :, None].to_broadcast([P, NJT, BW])
        mj_b = m_tabc[:, :, None].to_broadcast([P, NJT, BW])
        bm_b = bm16[:, None, :].to_broadcast([P, NJT, BW])
        Abd_t = io.tile([P, NJT, BW], fp16)
        nc.vector.tensor_tensor(out=Abd_t[:], in0=t_rep3, in1=tj_b, op=Alu.is_lt)
        Vbd = io.tile([P, NJT, BW], fp16)
        nc.vector.scalar_tensor_tensor(
            out=Vbd[:], in0=Abd_t[:], scalar=0.0, in1=bm_b,
            op0=Alu.add, op1=Alu.mult, accum_out=partials[:, C_NBAND:C_NBAND + 1])
        Bbd = io.tile([P, NJT, BW], fp16)
        nc.vector.tensor_tensor(out=Bbd[:], in0=m_rep3, in1=mj_b, op=Alu.is_lt)
        junk = io.tile([P, NJT, BW], fp16)
        nc.vector.scalar_tensor_tensor(
            out=junk[:], in0=Vbd[:], scalar=0.0, in1=Bbd[:],
            op0=Alu.add, op1=Alu.mult, accum_out=partials[:, C_SBAND:C_SBAND + 1])

        # ---------- early finals (DVE, input-only) ----------
        ta = io.tile([P, RT], f32)
        nc.vector.tensor_scalar_add(out=ta[:], in0=tte, scalar1=EPS)
        nc.vector.reciprocal(out=fin[:, 32:48], in_=ta[:])
        nc.vector.tensor_tensor(out=other[:, 32:48], in0=fin[:, 32:48], in1=ttg,
                                op=Alu.mult)
        req = io.tile([P, RT, 5], f32)
        nc.vector.tensor_tensor(
            out=req[:], in0=iota5,
            in1=rtgtf[:, :, None].to_broadcast([P, RT, 5]), op=Alu.is_equal)
        reqw = io.tile([P, RT, 5], f32)
        nc.vector.tensor_tensor(out=reqw[:], in0=req[:], in1=riskl, op=Alu.mult)
        nc.vector.tensor_reduce(out=other[:, 16:32], in_=reqw[:], axis=AxX,
                                op=Alu.add)
        nc.vector.tensor_reduce(out=partials[:, C_UNC:C_UNC + 1], in_=unc,
                                axis=AxX, op=Alu.add)
        nc.vector.tensor_copy(out=other[:, 0:16], in_=lt16[:])

        # ---------- batched Ln + final sums ----------
        ln_bi = nc.scalar.activation(out=finl[:], in_=fin[:], func=Act.Ln,
                                     bias=epsb[:])
        add_dep_helper(ln_bi.ins, exp_bis[-1].ins, sync=True,
                       reason="Ln after exp stream (ACT table switch once)")
        dtile = io.tile([P, 48], f32)
        nc.vector.tensor_tensor(out=dtile[:], in0=finl[:], in1=other[:],
                                op=Alu.subtract)
        nc.vector.tensor_reduce(
            out=partials[:, C_CE:C_CE + 3],
            in_=dtile[:].rearrange("p (g r) -> p g r", r=RT),
            axis=AxX, op=Alu.add)

        # ---------- output assembly ----------
        nc.scalar.activation(out=prefsb[:, WMAX:2 * WMAX], in_=psS[:],
                             func=Act.Copy)
        nc.sync.dma_start(out=pref_h[:], in_=prefsb[:])
        psF = psum.tile([1, NOUT], f32)
        nc.tensor.matmul(out=psF[:], lhsT=dummy1[:], rhs=partials[:],
                         start=True, stop=True)
        outs = io.tile([1, NOUT], f32)
        nc.vector.tensor_copy(out=outs[:], in_=psF[:])
        nc.sync.dma_start(out=out_h[:], in_=outs[:])

    nc.finalize()
    return nc


def _get_nc():
    if "nc" not in _CACHE:
        _CACHE["nc"] = _build_nc()
    return _CACHE["nc"]


def _rt_layout(x):
    # [S] -> [P, RT] with (p, t) = x[t*128 + p]
    return np.ascontiguousarray(x.reshape(RT, P).T)


def _rank_keys(t):
    # strictly increasing fp16-exact enumeration of the sorted order of t
    n = t.shape[0]
    order = np.argsort(t, kind="stable")
    ranks = np.empty(n, dtype=np.int64)
    ranks[order] = np.arange(n)
    e, m = np.divmod(ranks, 1024)
    return ((2.0 ** e) * (1.0 + m / 1024.0)).astype(np.float16)


def build_in_maps(disease_logits, disease_targets, time_to_event, time_targets,
                  risk_stratification, risk_targets, survival_curves,
                  survival_targets, event_indicators, uncertainty):
    f32 = np.float32
    disease_logits = np.asarray(disease_logits).astype(np.float16)
    disease_targets = np.asarray(disease_targets).astype(np.int64)
    time_to_event = np.asarray(time_to_event, f32)
    time_targets = np.asarray(time_targets, f32)
    risk_stratification = np.asarray(risk_stratification, f32)
    risk_targets = np.asarray(risk_targets).astype(np.int64)
    survival_curves = np.asarray(survival_curves).astype(np.float16)
    survival_targets = np.asarray(survival_targets, f32)
    event_indicators = np.asarray(event_indicators)
    uncertainty = np.asarray(uncertainty, f32)

    keys = _rank_keys(survival_targets)                       # [n] fp16 exact
    keytab = np.ascontiguousarray(keys.reshape(NJT, P).T)     # [P, NJT]
    curves_tab = np.ascontiguousarray(
        survival_curves.reshape(NJT, P, T_SURV).transpose(1, 0, 2)
    ).reshape(P, NJT * T_SURV)
    iota5 = np.broadcast_to(np.arange(5, dtype=f32), (P, RT, 5)).reshape(P, 80)
    row_base = np.arange(S, dtype=np.int64) * VOCAB
    in_maps = []
    for c in range(NCORES):
        tgt = disease_targets[c]
        gidx = (row_base + np.clip(tgt, 0, VOCAB - 1)).astype(np.int32)
        bmask = ((8 * np.arange(BW)[None, :] + c) < np.arange(P)[:, None])
        keyf = np.where(event_indicators[c::NCORES] == 1,
                        keys[c::NCORES], BIGKEY).astype(np.float16)
        smalls = np.concatenate([
            risk_targets[c].astype(f32).reshape(P, RT),             # O_RTGT
            time_to_event[c].reshape(P, RT),                        # O_TTE
            time_targets[c].reshape(P, RT),                         # O_TTG
            uncertainty[c].reshape(P, RT),                          # O_UNC
            iota5,                                                  # O_IOTA5
            risk_stratification[c].reshape(P, 80),                  # O_RISKL
            keytab.astype(f32),                                     # O_KEYTAB
        ], axis=1)
        smalls16 = bmask.astype(np.float16)
        assert smalls.shape == (P, SMALLW)
        assert smalls16.shape == (P, SMALL16W)
        in_maps.append({
            "logits": np.ascontiguousarray(disease_logits[c]),
            "gidx": _rt_layout(gidx),
            "smalls": np.ascontiguousarray(smalls),
            "smalls16": np.ascontiguousarray(smalls16),
            "keyf": keyf.reshape(1, W),
            "curvo": np.ascontiguousarray(
                survival_curves[c::NCORES].reshape(P, 4 * T_SURV)),
            "curves": curves_tab,
        })
    return in_maps


def combine(parts, prefs):
    # parts: [NCORES, NOUT]; prefs: [NCORES, 2*WMAX] per-core partial sums
    tot = parts.astype(np.float64).sum(axis=0)
    ptot = prefs.astype(np.float64).sum(axis=0)
    n_elem = float(B * S)
    disease = tot[C_CE] / n_elem
    risk = tot[C_RISK] / n_elem
    time_loss = -tot[C_TIME] / n_elem
    unc = tot[C_UNC] / n_elem * 0.01
    n_pairs = tot[C_NBAND] + ptot[0:WMAX].sum()
    s_conc = tot[C_SBAND] + ptot[WMAX:2 * WMAX].sum()
    if n_pairs > 0:
        surv = 1.0 - s_conc / max(n_pairs, 1.0)
    else:
        surv = 0.0
    total = disease + time_loss + risk + surv + unc
    return np.array([disease, time_loss, risk, surv, unc, total], dtype=np.float32)


def run_spmd(in_maps, **kw):
    from concourse.bass_utils import run_bass_kernel_spmd
    return run_bass_kernel_spmd(_get_nc(), in_maps, list(range(NCORES)), **kw)


def kernel(**inputs):
    in_maps = build_in_maps(**inputs)
    res = run_spmd(in_maps)
    parts = np.stack([res.results[c]["partials"].reshape(NOUT)
                      for c in range(NCORES)])
    prefs = np.stack([res.results[c]["prefix"].reshape(2 * WMAX)
                      for c in range(NCORES)])
    return combine(parts, prefs)

